# revision 6
# baseline (speedup 1.0000x reference)
"""CorrespondenceNet fused kernel for Trainium2 (8 NeuronCores, SPMD).

Math (reference):
    xf = colnorm(x); yf = colnorm(y)          # mean-center + L2-normalize each
                                              # channel column over the N rows
    corr = xf @ yf.T                          # [N, N]
    attn = softmax(corr / tau, axis=1)
    W = (attn @ ref_ab.reshape(2,-1).T).T.reshape(2, h, w)
    S = corr.max(axis=1).reshape(h, w)

Strategy: shard the N=16384 query rows of x across 8 cores (2048 rows each).
Each core computes its correlation slab fused in SBUF/PSUM (corr never touches
HBM), in "corrT" layout: j (y positions) on partitions, x rows on the free dim.
  - corr block  = matmul(lhsT=yhat j-tile [C,128], rhs=xhat chunk [C,512]) f32r
  - e = exp(corr/tau)     ACT, fp16 out (softmax max-subtraction cancels; the
                          exponent range is only ~±0.45 so no overflow)
  - warp + denominator    = matmul(lhsT=[ref_a, ref_b, 1, 0] fp16, rhs=e),
                          PSUM-accumulated over all 128 j-tiles
  - confidence            = tau * ln(max e)  (running tensor_max over e, then
                          gpsimd partition_all_reduce across the j partitions)
Normalization is folded as xhat = x - mu_x (unscaled) and
yhat = (y - mu_y) / (sx * sy), so corr == xf @ yf.T exactly.
Column stats come from bn_stats/bn_aggr over the full (replicated) transposed
inputs; y is streamed twice (stats pass, then normalize pass) so the f32r
operand tiles are produced by a compute op, as the BIR verifier requires.
"""

import numpy as np
from contextlib import ExitStack

import concourse.tile as tile
from concourse import bacc, mybir, bass_isa
from concourse.bass_utils import run_bass_kernel_spmd

F32 = mybir.dt.float32
F32R = mybir.dt.float32r
F16 = mybir.dt.float16
AF = mybir.ActivationFunctionType
ALU = mybir.AluOpType

N = 16384          # total rows (H*W)
C = 256            # channels
NCORES = 8
SLAB = N // NCORES  # 2048 x-rows per core
CHUNK = 1024        # x-rows per j-sweep (2 sweeps per core)
PIECE = 2048        # streaming piece width (columns of the [256, N] layouts)
TAU = 0.01

_CACHE = {}


def _build():
    nc = bacc.Bacc("TRN2", target_bir_lowering=False, debug=False)

    xt = nc.dram_tensor("xt", [C, N], F32, kind="ExternalInput").ap()
    yt = nc.dram_tensor("yt", [C, N], F32, kind="ExternalInput").ap()
    xslab = nc.dram_tensor("xslab", [C, SLAB], F32, kind="ExternalInput").ap()
    ref4 = nc.dram_tensor("ref4", [128, 512], F32, kind="ExternalInput").ap()
    w_out = nc.dram_tensor("w_out", [2, SLAB], F32, kind="ExternalOutput").ap()
    s_out = nc.dram_tensor("s_out", [1, SLAB], F32, kind="ExternalOutput").ap()

    n_pieces = N // PIECE  # 8 per channel-half

    with tile.TileContext(nc) as tc, ExitStack() as ctx:
        trans = ctx.enter_context(tc.tile_pool(name="trans", bufs=2))
        res = ctx.enter_context(tc.tile_pool(name="res", bufs=1))
        epool = ctx.enter_context(tc.tile_pool(name="e", bufs=3))
        rmpool = ctx.enter_context(tc.tile_pool(name="rm", bufs=2))
        small = ctx.enter_context(tc.tile_pool(name="small", bufs=1))
        cpsum = ctx.enter_context(tc.tile_pool(name="cpsum", bufs=3, space="PSUM"))
        wpsum = ctx.enter_context(tc.tile_pool(name="wpsum", bufs=1, space="PSUM"))

        # --- ref weights: [a, b, 1, 0] per j, tiled [128, 128*4], to fp16
        ref_f = trans.tile([128, 512], F32, tag="stage", name="ref_f")
        nc.sync.dma_start(ref_f[:], ref4[:])
        ref_h = small.tile([128, 512], F16, tag="ref_h", name="ref_h")
        nc.vector.tensor_copy(ref_h[:], ref_f[:])

        # --- stats pass: stream x then y, bn_stats each piece
        bnx = [small.tile([128, n_pieces * 24], F32, tag=f"bnx{c}", name=f"bnx{c}") for c in (0, 1)]
        bny = [small.tile([128, n_pieces * 24], F32, tag=f"bny{c}", name=f"bny{c}") for c in (0, 1)]
        for src, bnbuf in ((xt, bnx), (yt, bny)):
            for c in (0, 1):
                for i in range(n_pieces):
                    st = trans.tile([128, PIECE], F32, tag="stage", name="stage")
                    nc.sync.dma_start(
                        st[:], src[c * 128:(c + 1) * 128, i * PIECE:(i + 1) * PIECE])
                    for g in range(PIECE // 512):
                        nc.vector.bn_stats(
                            bnbuf[c][:, (i * 4 + g) * 6:(i * 4 + g + 1) * 6],
                            st[:, g * 512:(g + 1) * 512])

        # --- aggregate stats and build scales
        # mv*: [:, 0] = mean, [:, 1] = population variance
        mvx = [small.tile([128, 2], F32, tag=f"mvx{c}", name=f"mvx{c}") for c in (0, 1)]
        mvy = [small.tile([128, 2], F32, tag=f"mvy{c}", name=f"mvy{c}") for c in (0, 1)]
        aa = []
        for c in (0, 1):
            nc.vector.bn_aggr(mvx[c][:], bnx[c][:])
            nc.vector.bn_aggr(mvy[c][:], bny[c][:])
            # a = 1 / (N * sqrt(vx * vy));  norm_x * norm_y = N * sqrt(vx*vy)
            p = small.tile([128, 1], F32, tag=f"p{c}", name=f"p{c}")
            nc.vector.tensor_mul(p[:], mvx[c][:, 1:2], mvy[c][:, 1:2])
            r = small.tile([128, 1], F32, tag=f"r{c}", name=f"r{c}")
            nc.scalar.sqrt(r[:], p[:])
            # one Newton step: r2 = 0.5*(r + p/r), guards loose ACT sqrt ULPs
            rec = small.tile([128, 1], F32, tag=f"rec{c}", name=f"rec{c}")
            nc.vector.reciprocal(rec[:], r[:])
            t2 = small.tile([128, 1], F32, tag=f"t2{c}", name=f"t2{c}")
            nc.vector.tensor_mul(t2[:], p[:], rec[:])
            r2 = small.tile([128, 1], F32, tag=f"r2{c}", name=f"r2{c}")
            nc.vector.tensor_add(r2[:], r[:], t2[:])
            s = small.tile([128, 1], F32, tag=f"s{c}", name=f"s{c}")
            nc.vector.tensor_scalar_mul(s[:], r2[:], 0.5 * float(N))
            a = small.tile([128, 1], F32, tag=f"a{c}", name=f"a{c}")
            nc.vector.reciprocal(a[:], s[:])
            aa.append(a)

        # --- xhat = x - mu_x (f32r)
        xh = []
        for c in (0, 1):
            stg = trans.tile([128, SLAB], F32, tag="stage", name="xstage")
            nc.sync.dma_start(stg[:], xslab[c * 128:(c + 1) * 128, :])
            t = res.tile([128, SLAB], F32R, tag=f"xh{c}", name=f"xh{c}")
            nc.vector.tensor_scalar(
                out=t[:], in0=stg[:], scalar1=mvx[c][:, 0:1], scalar2=None,
                op0=ALU.subtract)
            xh.append(t)

        # --- yhat = (y - mu_y) * a (f32r), second streaming pass
        yh = [[None] * n_pieces, [None] * n_pieces]
        for c in (0, 1):
            for i in range(n_pieces):
                st = trans.tile([128, PIECE], F32, tag="stage", name="stage")
                nc.sync.dma_start(
                    st[:], yt[c * 128:(c + 1) * 128, i * PIECE:(i + 1) * PIECE])
                t = res.tile([128, PIECE], F32R, tag=f"yh{c}_{i}", name=f"yh{c}_{i}")
                nc.vector.tensor_scalar(
                    out=t[:], in0=st[:], scalar1=mvy[c][:, 0:1], scalar2=aa[c],
                    op0=ALU.subtract, op1=ALU.mult)
                yh[c][i] = t

        # --- main fused sweep
        jt_per_piece = PIECE // 128
        pms = []
        for h in range(SLAB // CHUNK):
            rm = rmpool.tile([128, CHUNK], F16, tag="rm", name="rm")
            nc.vector.memset(rm[:], 0.0)
            wa = wpsum.tile([4, CHUNK], F32, tag="wa", name="wa")
            for t in range(N // 128):
                ip, off = t // jt_per_piece, (t % jt_per_piece) * 128
                cp = cpsum.tile([128, CHUNK], F32, tag="corr", name="corr")
                for c in (0, 1):
                    lhsT = yh[c][ip][:, off:off + 128]
                    for q in (0, 1):
                        nc.tensor.matmul(
                            cp[:, q * 512:(q + 1) * 512], lhsT,
                            xh[c][:, h * CHUNK + q * 512:h * CHUNK + q * 512 + 512],
                            start=(c == 0), stop=(c == 1))
                et = epool.tile([128, CHUNK], F16, tag="e", name="e")
                nc.scalar.activation(et[:], cp[:], AF.Exp, scale=1.0 / TAU)
                nc.vector.tensor_max(rm[:], rm[:], et[:])
                for q in (0, 1):
                    nc.tensor.matmul(
                        wa[:, q * 512:(q + 1) * 512], ref_h[:, t * 4:t * 4 + 4],
                        et[:, q * 512:(q + 1) * 512],
                        start=(t == 0), stop=(t == N // 128 - 1))

            # per-chunk epilogue (no ACT table switches: Copy is in every set)
            pm = small.tile([128, CHUNK], F16, tag=f"pm{h}", name=f"pm{h}")
            nc.gpsimd.partition_all_reduce(
                pm[:], rm[:], channels=128, reduce_op=bass_isa.ReduceOp.max)
            pms.append(pm)
            wsb = small.tile([4, CHUNK], F32, tag="wsb", name="wsb")
            nc.scalar.copy(wsb[:], wa[:])
            dt = small.tile([1, CHUNK], F32, tag="dt", name="dt")
            nc.sync.dma_start(dt[:], wsb[2:3, :])           # denominator -> part 0
            den = small.tile([1, CHUNK], F32, tag="den", name="den")
            nc.vector.reciprocal(den[:], dt[:])
            denb = small.tile([2, CHUNK], F32, tag="denb", name="denb")
            nc.sync.dma_start(denb[0:1, :], den[0:1, :])
            nc.sync.dma_start(denb[1:2, :], den[0:1, :])
            nc.vector.tensor_mul(wsb[0:2, :], wsb[0:2, :], denb[:])
            nc.sync.dma_start(w_out[:, h * CHUNK:(h + 1) * CHUNK], wsb[0:2, :])

        # --- confidence: tau * ln(max e), done last (one ACT table switch)
        for h, pm in enumerate(pms):
            cf = small.tile([1, CHUNK], F32, tag="cf", name="cf")
            nc.scalar.activation(cf[:], pm[0:1, :], AF.Ln)
            nc.vector.tensor_scalar_mul(cf[:], cf[:], TAU)
            nc.sync.dma_start(s_out[:, h * CHUNK:(h + 1) * CHUNK], cf[:])

    nc.compile()
    return nc


def _get_nc():
    if "nc" not in _CACHE:
        _CACHE["nc"] = _build()
    return _CACHE["nc"]


def kernel(x_feature, y_feature, ref_ab, h, w, **_unused):
    x = np.ascontiguousarray(np.asarray(x_feature, dtype=np.float32))
    y = np.ascontiguousarray(np.asarray(y_feature, dtype=np.float32))
    r = np.asarray(ref_ab, dtype=np.float32).reshape(2, -1)
    hh, ww = int(h), int(w)
    assert x.shape == (N, C) and y.shape == (N, C) and r.shape == (2, N)

    xt = np.ascontiguousarray(x.T)
    yt = np.ascontiguousarray(y.T)

    r4 = np.zeros((N, 4), np.float32)
    r4[:, 0] = r[0]
    r4[:, 1] = r[1]
    r4[:, 2] = 1.0
    ref4 = np.ascontiguousarray(
        r4.reshape(N // 128, 128, 4).transpose(1, 0, 2).reshape(128, 512))

    nc = _get_nc()
    in_maps = []
    for k in range(NCORES):
        in_maps.append({
            "xt": xt,
            "yt": yt,
            "xslab": np.ascontiguousarray(xt[:, k * SLAB:(k + 1) * SLAB]),
            "ref4": ref4,
        })
    bres = run_bass_kernel_spmd(nc, in_maps, core_ids=list(range(NCORES)))

    W = np.empty((2, N), np.float32)
    S = np.empty((N,), np.float32)
    for k in range(NCORES):
        W[:, k * SLAB:(k + 1) * SLAB] = bres.results[k]["w_out"]
        S[k * SLAB:(k + 1) * SLAB] = bres.results[k]["s_out"][0]
    return (W.reshape(2, hh, ww), S.reshape(hh, ww))


# revision 7
# speedup vs baseline: 11961.4002x; 11961.4002x over previous
"""CorrespondenceNet fused kernel for Trainium2 (8 NeuronCores, SPMD).

Math (reference):
    xf = colnorm(x); yf = colnorm(y)          # mean-center + L2-normalize each
                                              # channel column over the N rows
    corr = xf @ yf.T                          # [N, N]
    attn = softmax(corr / tau, axis=1)
    W = (attn @ ref_ab.reshape(2,-1).T).T.reshape(2, h, w)
    S = corr.max(axis=1).reshape(h, w)

Strategy: shard the N=16384 query rows of x across 8 cores (2048 rows each).
Each core computes its correlation slab fused in SBUF/PSUM (corr never touches
HBM), in "corrT" layout: j (y positions) on partitions, x rows on the free dim.
  - corr block  = matmul(lhsT=yhat j-tile [C,128], rhs=xhat chunk [C,512]) f32r
  - e = exp(corr/tau)     ACT, fp16 out (softmax max-subtraction cancels; the
                          exponent range is only ~±0.45 so no overflow)
  - warp + denominator    = matmul(lhsT=[ref_a, ref_b, 1, 0] fp16, rhs=e),
                          PSUM-accumulated over all 128 j-tiles
  - confidence            = tau * ln(max e)  (running tensor_max over e, then
                          gpsimd partition_all_reduce across the j partitions)
Normalization is folded as xhat = x - mu_x (unscaled) and
yhat = (y - mu_y) / (sx * sy), so corr == xf @ yf.T exactly.
Column stats come from bn_stats/bn_aggr over the full (replicated) transposed
inputs; y is streamed twice (stats pass, then normalize pass) so the f32r
operand tiles are produced by a compute op, as the BIR verifier requires.
"""

import numpy as np
from contextlib import ExitStack

import concourse.tile as tile
from concourse import bacc, mybir, bass_isa
from concourse.bass_utils import run_bass_kernel_spmd

F32 = mybir.dt.float32
F32R = mybir.dt.float32r
F16 = mybir.dt.float16
AF = mybir.ActivationFunctionType
ALU = mybir.AluOpType

N = 16384          # total rows (H*W)
C = 256            # channels
NCORES = 8
SLAB = N // NCORES  # 2048 x-rows per core
CHUNK = 1024        # x-rows per j-sweep (2 sweeps per core)
PIECE = 2048        # streaming piece width (columns of the [256, N] layouts)
TAU = 0.01

_CACHE = {}


def _build():
    nc = bacc.Bacc("TRN2", target_bir_lowering=False, debug=False)

    xt = nc.dram_tensor("xt", [C, N], F32, kind="ExternalInput").ap()
    yt = nc.dram_tensor("yt", [C, N], F32, kind="ExternalInput").ap()
    xslab = nc.dram_tensor("xslab", [C, SLAB], F32, kind="ExternalInput").ap()
    ref4 = nc.dram_tensor("ref4", [128, 512], F32, kind="ExternalInput").ap()
    w_out = nc.dram_tensor("w_out", [2, SLAB], F32, kind="ExternalOutput").ap()
    s_out = nc.dram_tensor("s_out", [1, SLAB], F32, kind="ExternalOutput").ap()

    n_pieces = N // PIECE  # 8 per channel-half

    with tile.TileContext(nc) as tc, ExitStack() as ctx:
        trans = ctx.enter_context(tc.tile_pool(name="trans", bufs=2))
        res = ctx.enter_context(tc.tile_pool(name="res", bufs=1))
        epool = ctx.enter_context(tc.tile_pool(name="e", bufs=3))
        rmpool = ctx.enter_context(tc.tile_pool(name="rm", bufs=2))
        small = ctx.enter_context(tc.tile_pool(name="small", bufs=1))
        cpsum = ctx.enter_context(tc.tile_pool(name="cpsum", bufs=3, space="PSUM"))
        wpsum = ctx.enter_context(tc.tile_pool(name="wpsum", bufs=1, space="PSUM"))

        # --- ref weights: [a, b, 1, 0] per j, tiled [128, 128*4], to fp16
        ref_f = trans.tile([128, 512], F32, tag="xstg", name="ref_f")
        nc.sync.dma_start(ref_f[:], ref4[:])
        ref_h = small.tile([128, 512], F16, tag="ref_h", name="ref_h")
        nc.vector.tensor_copy(ref_h[:], ref_f[:])

        # --- stats pass: stream x then y, bn_stats each piece
        bnx = [small.tile([128, n_pieces * 24], F32, tag=f"bnx{c}", name=f"bnx{c}") for c in (0, 1)]
        bny = [small.tile([128, n_pieces * 24], F32, tag=f"bny{c}", name=f"bny{c}") for c in (0, 1)]
        STP = 1024
        for src_ap, bnbuf, stag in ((xt, bnx, "xstg"), (yt, bny, "ystg")):
            for c in (0, 1):
                for i in range(N // STP):
                    st = trans.tile([128, STP], F32, tag=stag, name="stage")
                    nc.sync.dma_start(
                        st[:], src_ap[c * 128:(c + 1) * 128, i * STP:(i + 1) * STP])
                    for g in range(STP // 512):
                        nc.vector.bn_stats(
                            bnbuf[c][:, (i * 2 + g) * 6:(i * 2 + g + 1) * 6],
                            st[:, g * 512:(g + 1) * 512])

        # --- aggregate stats and build scales
        # mv*: [:, 0] = mean, [:, 1] = population variance
        mvx = [small.tile([128, 2], F32, tag=f"mvx{c}", name=f"mvx{c}") for c in (0, 1)]
        mvy = [small.tile([128, 2], F32, tag=f"mvy{c}", name=f"mvy{c}") for c in (0, 1)]
        aa = []
        for c in (0, 1):
            nc.vector.bn_aggr(mvx[c][:], bnx[c][:])
            nc.vector.bn_aggr(mvy[c][:], bny[c][:])
            # a = 1 / (N * sqrt(vx * vy));  norm_x * norm_y = N * sqrt(vx*vy)
            p = small.tile([128, 1], F32, tag=f"p{c}", name=f"p{c}")
            nc.vector.tensor_mul(p[:], mvx[c][:, 1:2], mvy[c][:, 1:2])
            r = small.tile([128, 1], F32, tag=f"r{c}", name=f"r{c}")
            nc.scalar.sqrt(r[:], p[:])
            # one Newton step: r2 = 0.5*(r + p/r), guards loose ACT sqrt ULPs
            rec = small.tile([128, 1], F32, tag=f"rec{c}", name=f"rec{c}")
            nc.vector.reciprocal(rec[:], r[:])
            t2 = small.tile([128, 1], F32, tag=f"t2{c}", name=f"t2{c}")
            nc.vector.tensor_mul(t2[:], p[:], rec[:])
            r2 = small.tile([128, 1], F32, tag=f"r2{c}", name=f"r2{c}")
            nc.vector.tensor_add(r2[:], r[:], t2[:])
            s = small.tile([128, 1], F32, tag=f"s{c}", name=f"s{c}")
            nc.vector.tensor_scalar_mul(s[:], r2[:], 0.5 * float(N))
            a = small.tile([128, 1], F32, tag=f"a{c}", name=f"a{c}")
            nc.vector.reciprocal(a[:], s[:])
            aa.append(a)

        # --- xhat = x - mu_x (f32r)
        xh = []
        for c in (0, 1):
            t = res.tile([128, SLAB], F32R, tag=f"xh{c}", name=f"xh{c}")
            for u in range(SLAB // 1024):
                stg = trans.tile([128, 1024], F32, tag="xstg", name="xstage")
                nc.sync.dma_start(
                    stg[:], xslab[c * 128:(c + 1) * 128, u * 1024:(u + 1) * 1024])
                nc.vector.tensor_scalar(
                    out=t[:, u * 1024:(u + 1) * 1024], in0=stg[:],
                    scalar1=mvx[c][:, 0:1], scalar2=None, op0=ALU.subtract)
            xh.append(t)

        # --- yhat = (y - mu_y) * a (f32r), second streaming pass
        yh = [[None] * n_pieces, [None] * n_pieces]
        for c in (0, 1):
            for i in range(n_pieces):
                t = res.tile([128, PIECE], F32R, tag=f"yh{c}_{i}", name=f"yh{c}_{i}")
                for u in range(PIECE // 1024):
                    st = trans.tile([128, 1024], F32, tag="ystg", name="stage")
                    nc.sync.dma_start(
                        st[:], yt[c * 128:(c + 1) * 128,
                                  i * PIECE + u * 1024:i * PIECE + (u + 1) * 1024])
                    nc.vector.tensor_scalar(
                        out=t[:, u * 1024:(u + 1) * 1024], in0=st[:],
                        scalar1=mvy[c][:, 0:1], scalar2=aa[c],
                        op0=ALU.subtract, op1=ALU.mult)
                yh[c][i] = t

        # --- main fused sweep
        jt_per_piece = PIECE // 128
        pms = []
        for h in range(SLAB // CHUNK):
            rm = rmpool.tile([128, CHUNK], F16, tag="rm", name="rm")
            nc.vector.memset(rm[:], 0.0)
            wa = wpsum.tile([4, CHUNK], F32, tag="wa", name="wa")
            for t in range(N // 128):
                ip, off = t // jt_per_piece, (t % jt_per_piece) * 128
                cp = cpsum.tile([128, CHUNK], F32, tag="corr", name="corr")
                for c in (0, 1):
                    lhsT = yh[c][ip][:, off:off + 128]
                    for q in (0, 1):
                        nc.tensor.matmul(
                            cp[:, q * 512:(q + 1) * 512], lhsT,
                            xh[c][:, h * CHUNK + q * 512:h * CHUNK + q * 512 + 512],
                            start=(c == 0), stop=(c == 1))
                et = epool.tile([128, CHUNK], F16, tag="e", name="e")
                nc.scalar.activation(et[:], cp[:], AF.Exp, scale=1.0 / TAU)
                nc.vector.tensor_max(rm[:], rm[:], et[:])
                for q in (0, 1):
                    nc.tensor.matmul(
                        wa[:, q * 512:(q + 1) * 512], ref_h[:, t * 4:t * 4 + 4],
                        et[:, q * 512:(q + 1) * 512],
                        start=(t == 0), stop=(t == N // 128 - 1))

            # per-chunk epilogue (no ACT table switches: Copy is in every set)
            pm = small.tile([128, CHUNK], F16, tag=f"pm{h}", name=f"pm{h}")
            nc.gpsimd.partition_all_reduce(
                pm[:], rm[:], channels=128, reduce_op=bass_isa.ReduceOp.max)
            pms.append(pm)
            wsb = small.tile([4, CHUNK], F32, tag="wsb", name="wsb")
            nc.scalar.copy(wsb[:], wa[:])
            dt = small.tile([1, CHUNK], F32, tag="dt", name="dt")
            nc.sync.dma_start(dt[:], wsb[2:3, :])           # denominator -> part 0
            den = small.tile([1, CHUNK], F32, tag="den", name="den")
            nc.vector.reciprocal(den[:], dt[:])
            denb = small.tile([2, CHUNK], F32, tag="denb", name="denb")
            nc.sync.dma_start(denb[0:1, :], den[0:1, :])
            nc.sync.dma_start(denb[1:2, :], den[0:1, :])
            nc.vector.tensor_mul(wsb[0:2, :], wsb[0:2, :], denb[:])
            nc.sync.dma_start(w_out[:, h * CHUNK:(h + 1) * CHUNK], wsb[0:2, :])

        # --- confidence: tau * ln(max e), done last (one ACT table switch)
        for h, pm in enumerate(pms):
            cf = small.tile([1, CHUNK], F32, tag="cf", name="cf")
            nc.scalar.activation(cf[:], pm[0:1, :], AF.Ln)
            nc.vector.tensor_scalar_mul(cf[:], cf[:], TAU)
            nc.sync.dma_start(s_out[:, h * CHUNK:(h + 1) * CHUNK], cf[:])

    nc.compile()
    return nc


def _get_nc():
    if "nc" not in _CACHE:
        _CACHE["nc"] = _build()
    return _CACHE["nc"]


def kernel(x_feature, y_feature, ref_ab, h, w, **_unused):
    x = np.ascontiguousarray(np.asarray(x_feature, dtype=np.float32))
    y = np.ascontiguousarray(np.asarray(y_feature, dtype=np.float32))
    r = np.asarray(ref_ab, dtype=np.float32).reshape(2, -1)
    hh, ww = int(h), int(w)
    assert x.shape == (N, C) and y.shape == (N, C) and r.shape == (2, N)

    xt = np.ascontiguousarray(x.T)
    yt = np.ascontiguousarray(y.T)

    r4 = np.zeros((N, 4), np.float32)
    r4[:, 0] = r[0]
    r4[:, 1] = r[1]
    r4[:, 2] = 1.0
    ref4 = np.ascontiguousarray(
        r4.reshape(N // 128, 128, 4).transpose(1, 0, 2).reshape(128, 512))

    nc = _get_nc()
    in_maps = []
    for k in range(NCORES):
        in_maps.append({
            "xt": xt,
            "yt": yt,
            "xslab": np.ascontiguousarray(xt[:, k * SLAB:(k + 1) * SLAB]),
            "ref4": ref4,
        })
    bres = run_bass_kernel_spmd(nc, in_maps, core_ids=list(range(NCORES)))

    W = np.empty((2, N), np.float32)
    S = np.empty((N,), np.float32)
    for k in range(NCORES):
        W[:, k * SLAB:(k + 1) * SLAB] = bres.results[k]["w_out"]
        S[k * SLAB:(k + 1) * SLAB] = bres.results[k]["s_out"][0]
    return (W.reshape(2, hh, ww), S.reshape(hh, ww))


# revision 8
# speedup vs baseline: 13332.5222x; 1.1146x over previous
"""CorrespondenceNet fused kernel for Trainium2 (8 NeuronCores, SPMD).

Math (reference):
    xf = colnorm(x); yf = colnorm(y)          # mean-center + L2-normalize each
                                              # channel column over the N rows
    corr = xf @ yf.T                          # [N, N]
    attn = softmax(corr / tau, axis=1)
    W = (attn @ ref_ab.reshape(2,-1).T).T.reshape(2, h, w)
    S = corr.max(axis=1).reshape(h, w)

Strategy: shard the N=16384 query rows of x across 8 cores (2048 rows each).
Each core computes its correlation slab fused in SBUF/PSUM (corr never touches
HBM), in "corrT" layout: j (y positions) on partitions, x rows on the free dim.
  - corr block  = matmul(lhsT=yhat j-tile [C,128], rhs=xhat chunk [C,512]) f32r
  - e = exp(corr/tau)     ACT, fp16 out (softmax max-subtraction cancels; the
                          exponent range is only ~±0.45 so no overflow)
  - warp + denominator    = matmul(lhsT=[ref_a, ref_b, 1, 0] fp16, rhs=e),
                          PSUM-accumulated over all 128 j-tiles
  - confidence            = tau * ln(max e)  (running tensor_max over e, then
                          gpsimd partition_all_reduce across the j partitions)
Normalization is folded as xhat = x - mu_x (unscaled) and
yhat = (y - mu_y) / (sx * sy), so corr == xf @ yf.T exactly.
Column stats come from bn_stats/bn_aggr over the full (replicated) transposed
inputs; y is streamed twice (stats pass, then normalize pass) so the f32r
operand tiles are produced by a compute op, as the BIR verifier requires.
"""

import numpy as np
from contextlib import ExitStack

import concourse.tile as tile
from concourse import bacc, mybir, bass_isa
from concourse.bass_utils import run_bass_kernel_spmd

F32 = mybir.dt.float32
F32R = mybir.dt.float32r
F16 = mybir.dt.float16
AF = mybir.ActivationFunctionType
ALU = mybir.AluOpType

N = 16384          # total rows (H*W)
C = 256            # channels
NCORES = 8
SLAB = N // NCORES  # 2048 x-rows per core
CHUNK = 1024        # x-rows per j-sweep (2 sweeps per core)
PIECE = 2048        # streaming piece width (columns of the [256, N] layouts)
TAU = 0.01

_CACHE = {}


def _build():
    nc = bacc.Bacc("TRN2", target_bir_lowering=False, debug=False, num_devices=NCORES)

    yt = nc.dram_tensor("yt", [C, N], F32, kind="ExternalInput").ap()
    xslab = nc.dram_tensor("xslab", [C, SLAB], F32, kind="ExternalInput").ap()
    ref4 = nc.dram_tensor("ref4", [128, 512], F32, kind="ExternalInput").ap()
    w_out = nc.dram_tensor("w_out", [2, SLAB], F32, kind="ExternalOutput").ap()
    s_out = nc.dram_tensor("s_out", [1, SLAB], F32, kind="ExternalOutput").ap()
    ccin = nc.dram_tensor("ccin", [128, 4], F32).ap()
    ccout = nc.dram_tensor("ccout", [128, 4], F32, addr_space="Shared").ap()

    n_pieces = N // PIECE  # 8 per channel-half

    with tile.TileContext(nc) as tc, ExitStack() as ctx:
        trans = ctx.enter_context(tc.tile_pool(name="trans", bufs=2))
        res = ctx.enter_context(tc.tile_pool(name="res", bufs=1))
        epool = ctx.enter_context(tc.tile_pool(name="e", bufs=3))
        rmpool = ctx.enter_context(tc.tile_pool(name="rm", bufs=2))
        small = ctx.enter_context(tc.tile_pool(name="small", bufs=1))
        cpsum = ctx.enter_context(tc.tile_pool(name="cpsum", bufs=3, space="PSUM"))
        wpsum = ctx.enter_context(tc.tile_pool(name="wpsum", bufs=1, space="PSUM"))

        # --- ref weights: [a, b, 1, 0] per j, tiled [128, 128*4], to fp16
        ref_f = trans.tile([128, 512], F32, tag="xstg", name="ref_f")
        nc.sync.dma_start(ref_f[:], ref4[:])
        ref_h = small.tile([128, 512], F16, tag="ref_h", name="ref_h")
        nc.vector.tensor_copy(ref_h[:], ref_f[:])

        # --- stats pass: stream x then y, bn_stats each piece
        bny = [small.tile([128, n_pieces * 24], F32, tag=f"bny{c}", name=f"bny{c}") for c in (0, 1)]
        STP = 1024
        for src_ap, bnbuf, stag in ((yt, bny, "ystg"),):
            for c in (0, 1):
                for i in range(N // STP):
                    st = trans.tile([128, STP], F32, tag=stag, name="stage")
                    nc.sync.dma_start(
                        st[:], src_ap[c * 128:(c + 1) * 128, i * STP:(i + 1) * STP])
                    for g in range(STP // 512):
                        nc.vector.bn_stats(
                            bnbuf[c][:, (i * 2 + g) * 6:(i * 2 + g + 1) * 6],
                            st[:, g * 512:(g + 1) * 512])

        # x stats: per-core partial sums over the slab, AllReduce across cores.
        # P layout [128, (stat s in c0sx,c0sq,c1sx,c1sq) x (piece u)] -> S4 sums
        P = small.tile([128, 16], F32, tag="ppart", name="ppart")
        for c in (0, 1):
            for u in range(SLAB // 1024):
                stg = trans.tile([128, 1024], F32, tag="xstg", name="xsstat")
                nc.sync.dma_start(
                    stg[:], xslab[c * 128:(c + 1) * 128, u * 1024:(u + 1) * 1024])
                scr = cpsum.tile([128, 1024], F32, tag="corr", name="sqscr")
                nc.scalar.activation(scr[:], stg[:], AF.Identity,
                                     accum_out=P[:, (2 * c) * 4 + u:(2 * c) * 4 + u + 1])
                scr2 = cpsum.tile([128, 1024], F32, tag="corr", name="sqscr2")
                nc.scalar.activation(scr2[:], stg[:], AF.Square,
                                     accum_out=P[:, (2 * c + 1) * 4 + u:(2 * c + 1) * 4 + u + 1])
        S4 = small.tile([128, 4], F32, tag="s4", name="s4")
        nc.vector.tensor_reduce(S4[:], P[:].rearrange("p (s u) -> p s u", u=4),
                                axis=mybir.AxisListType.X, op=ALU.add)
        nc.sync.dma_start(ccin[:], S4[:])
        nc.gpsimd.collective_compute(
            kind="AllReduce", op=ALU.add,
            replica_groups=[list(range(NCORES))], ins=[ccin[:]], outs=[ccout[:]])
        G = small.tile([128, 4], F32, tag="gstat", name="gstat")
        nc.sync.dma_start(G[:], ccout[:])
        mux, varx = [], []
        for c in (0, 1):
            m = small.tile([128, 1], F32, tag=f"mux{c}", name=f"mux{c}")
            nc.vector.tensor_scalar_mul(m[:], G[:, 2 * c:2 * c + 1], 1.0 / N)
            m2 = small.tile([128, 1], F32, tag=f"m2{c}", name=f"m2{c}")
            nc.vector.tensor_mul(m2[:], m[:], m[:])
            v = small.tile([128, 1], F32, tag=f"varx{c}", name=f"varx{c}")
            nc.vector.tensor_scalar_mul(v[:], G[:, 2 * c + 1:2 * c + 2], 1.0 / N)
            nc.vector.tensor_sub(v[:], v[:], m2[:])
            mux.append(m)
            varx.append(v)

        # --- aggregate stats and build scales
        # mv*: [:, 0] = mean, [:, 1] = population variance
        mvy = [small.tile([128, 2], F32, tag=f"mvy{c}", name=f"mvy{c}") for c in (0, 1)]
        aa = []
        for c in (0, 1):
            nc.vector.bn_aggr(mvy[c][:], bny[c][:])
            # a = 1 / (N * sqrt(vx * vy));  norm_x * norm_y = N * sqrt(vx*vy)
            p = small.tile([128, 1], F32, tag=f"p{c}", name=f"p{c}")
            nc.vector.tensor_mul(p[:], varx[c][:], mvy[c][:, 1:2])
            r = small.tile([128, 1], F32, tag=f"r{c}", name=f"r{c}")
            nc.scalar.sqrt(r[:], p[:])
            # one Newton step: r2 = 0.5*(r + p/r), guards loose ACT sqrt ULPs
            rec = small.tile([128, 1], F32, tag=f"rec{c}", name=f"rec{c}")
            nc.vector.reciprocal(rec[:], r[:])
            t2 = small.tile([128, 1], F32, tag=f"t2{c}", name=f"t2{c}")
            nc.vector.tensor_mul(t2[:], p[:], rec[:])
            r2 = small.tile([128, 1], F32, tag=f"r2{c}", name=f"r2{c}")
            nc.vector.tensor_add(r2[:], r[:], t2[:])
            s = small.tile([128, 1], F32, tag=f"s{c}", name=f"s{c}")
            nc.vector.tensor_scalar_mul(s[:], r2[:], 0.5 * float(N))
            a = small.tile([128, 1], F32, tag=f"a{c}", name=f"a{c}")
            nc.vector.reciprocal(a[:], s[:])
            aa.append(a)

        # --- xhat = x - mu_x (f32r)
        xh = []
        for c in (0, 1):
            t = res.tile([128, SLAB], F32R, tag=f"xh{c}", name=f"xh{c}")
            for u in range(SLAB // 1024):
                stg = trans.tile([128, 1024], F32, tag="xstg", name="xstage")
                nc.sync.dma_start(
                    stg[:], xslab[c * 128:(c + 1) * 128, u * 1024:(u + 1) * 1024])
                nc.vector.tensor_scalar(
                    out=t[:, u * 1024:(u + 1) * 1024], in0=stg[:],
                    scalar1=mux[c][:], scalar2=None, op0=ALU.subtract)
            xh.append(t)

        # --- yhat = (y - mu_y) * a (f32r), second streaming pass
        yh = [[None] * n_pieces, [None] * n_pieces]
        for c in (0, 1):
            for i in range(n_pieces):
                t = res.tile([128, PIECE], F32R, tag=f"yh{c}_{i}", name=f"yh{c}_{i}")
                for u in range(PIECE // 1024):
                    st = trans.tile([128, 1024], F32, tag="ystg", name="stage")
                    nc.sync.dma_start(
                        st[:], yt[c * 128:(c + 1) * 128,
                                  i * PIECE + u * 1024:i * PIECE + (u + 1) * 1024])
                    nc.vector.tensor_scalar(
                        out=t[:, u * 1024:(u + 1) * 1024], in0=st[:],
                        scalar1=mvy[c][:, 0:1], scalar2=aa[c],
                        op0=ALU.subtract, op1=ALU.mult)
                yh[c][i] = t

        # --- main fused sweep
        jt_per_piece = PIECE // 128
        pms = []
        for h in range(SLAB // CHUNK):
            rm = rmpool.tile([128, CHUNK], F16, tag="rm", name="rm")
            nc.vector.memset(rm[:], 0.0)
            wa = wpsum.tile([4, CHUNK], F32, tag="wa", name="wa")
            for t in range(N // 128):
                ip, off = t // jt_per_piece, (t % jt_per_piece) * 128
                cp = cpsum.tile([128, CHUNK], F32, tag="corr", name="corr")
                for c in (0, 1):
                    lhsT = yh[c][ip][:, off:off + 128]
                    for q in (0, 1):
                        nc.tensor.matmul(
                            cp[:, q * 512:(q + 1) * 512], lhsT,
                            xh[c][:, h * CHUNK + q * 512:h * CHUNK + q * 512 + 512],
                            start=(c == 0), stop=(c == 1))
                et = epool.tile([128, CHUNK], F16, tag="e", name="e")
                nc.scalar.activation(et[:], cp[:], AF.Exp, scale=1.0 / TAU)
                nc.vector.tensor_max(rm[:], rm[:], et[:])
                for q in (0, 1):
                    nc.tensor.matmul(
                        wa[:, q * 512:(q + 1) * 512], ref_h[:, t * 4:t * 4 + 4],
                        et[:, q * 512:(q + 1) * 512],
                        start=(t == 0), stop=(t == N // 128 - 1))

            # per-chunk epilogue (no ACT table switches: Copy is in every set)
            pm = small.tile([128, CHUNK], F16, tag=f"pm{h}", name=f"pm{h}")
            nc.gpsimd.partition_all_reduce(
                pm[:], rm[:], channels=128, reduce_op=bass_isa.ReduceOp.max)
            pms.append(pm)
            wsb = small.tile([4, CHUNK], F32, tag="wsb", name="wsb")
            nc.scalar.copy(wsb[:], wa[:])
            dt = small.tile([1, CHUNK], F32, tag="dt", name="dt")
            nc.sync.dma_start(dt[:], wsb[2:3, :])           # denominator -> part 0
            den = small.tile([1, CHUNK], F32, tag="den", name="den")
            nc.vector.reciprocal(den[:], dt[:])
            denb = small.tile([2, CHUNK], F32, tag="denb", name="denb")
            nc.sync.dma_start(denb[0:1, :], den[0:1, :])
            nc.sync.dma_start(denb[1:2, :], den[0:1, :])
            nc.vector.tensor_mul(wsb[0:2, :], wsb[0:2, :], denb[:])
            nc.sync.dma_start(w_out[:, h * CHUNK:(h + 1) * CHUNK], wsb[0:2, :])

        # --- confidence: tau * ln(max e), done last (one ACT table switch)
        for h, pm in enumerate(pms):
            cf = small.tile([1, CHUNK], F32, tag="cf", name="cf")
            nc.scalar.activation(cf[:], pm[0:1, :], AF.Ln)
            nc.vector.tensor_scalar_mul(cf[:], cf[:], TAU)
            nc.sync.dma_start(s_out[:, h * CHUNK:(h + 1) * CHUNK], cf[:])

    nc.compile()
    return nc


def _get_nc():
    if "nc" not in _CACHE:
        _CACHE["nc"] = _build()
    return _CACHE["nc"]


def kernel(x_feature, y_feature, ref_ab, h, w, **_unused):
    x = np.ascontiguousarray(np.asarray(x_feature, dtype=np.float32))
    y = np.ascontiguousarray(np.asarray(y_feature, dtype=np.float32))
    r = np.asarray(ref_ab, dtype=np.float32).reshape(2, -1)
    hh, ww = int(h), int(w)
    assert x.shape == (N, C) and y.shape == (N, C) and r.shape == (2, N)

    xt = np.ascontiguousarray(x.T)
    yt = np.ascontiguousarray(y.T)

    r4 = np.zeros((N, 4), np.float32)
    r4[:, 0] = r[0]
    r4[:, 1] = r[1]
    r4[:, 2] = 1.0
    ref4 = np.ascontiguousarray(
        r4.reshape(N // 128, 128, 4).transpose(1, 0, 2).reshape(128, 512))

    nc = _get_nc()
    in_maps = []
    for k in range(NCORES):
        in_maps.append({
            "yt": yt,
            "xslab": np.ascontiguousarray(xt[:, k * SLAB:(k + 1) * SLAB]),
            "ref4": ref4,
        })
    bres = run_bass_kernel_spmd(nc, in_maps, core_ids=list(range(NCORES)))

    W = np.empty((2, N), np.float32)
    S = np.empty((N,), np.float32)
    for k in range(NCORES):
        W[:, k * SLAB:(k + 1) * SLAB] = bres.results[k]["w_out"]
        S[k * SLAB:(k + 1) * SLAB] = bres.results[k]["s_out"][0]
    return (W.reshape(2, hh, ww), S.reshape(hh, ww))
